# revision 1
# baseline (speedup 1.0000x reference)
"""CoRoPE attention kernel for 8 trn2 NeuronCores (Bass/Tile).

Sharding: core c handles batch b = c//4 and heads 4*(c%4) .. 4*(c%4)+3
(data-parallel over batch x head-parallel, Megatron column/row split).
Each core computes its heads' full attention + a partial output
projection; the host sums the 4 partials per batch and adds bo.

Reference computation (B=2, S=2048, E=1024, H=16, D=64):
  q,k,v = x @ W{q,k,v}.T + b    -> (B,H,S,D)
  gates = sigmoid(sum(q*k, -1) * 1/sqrt(D));  a_k = cumsum(gates, seq)
  angles = a_k[...,None] * freqs;  q,k = interleaved-rope(q,k, angles)
  out = softmax_causal(q k^T/sqrt(D)) v  -> o @ Wo.T + bo

Device layout tricks:
  - all per-head tensors live transposed (feature on partitions, seq on
    free dim); host pre-transposes x and the weight slices.
  - RoPE pairs are de-interleaved by permuting W rows on the host, so
    the pair swap is a +-32-partition copy.
  - sm_scale is folded into Wq on the host.
  - biases are folded in via an appended ones-row on x^T and a bias row
    on the weights (skipped at build time when all biases are zero).
  - softmax row sums come from a ones column appended to V.
"""
import sys

sys.path.insert(0, "/opt/trn_rl_repo")

import math
import numpy as np

import concourse.bass as bass
import concourse.tile as tile
import concourse.mybir as mybir
from concourse import bacc
from concourse.bass_utils import run_bass_kernel_spmd

F32 = mybir.dt.float32
F32R = mybir.dt.float32r
I32 = mybir.dt.int32

B, S, E, H = 2, 2048, 1024, 16
D = E // H          # 64
NH = 4              # heads per core
NCORES = 8
THETA = 10000.0
SM = 1.0 / math.sqrt(D)
PI = math.pi
TWO_PI = 2.0 * math.pi

SLAB = 256          # query-column slab for attention (f32r needs N>=256)
NSLAB = S // SLAB   # 8
GROUP = 4           # key-blocks per PSUM exp-group (2 banks x 3 bufs)

# Precision plan: everything upstream of the gate cumsum (q/k
# projections, gate reduce) runs exact fp32 matmuls (4 cyc/row) because
# the cumsum amplifies per-gate error by ~sqrt(S).  Everything after the
# rope (QK^T, PV, out-proj) runs fp32r (1 cyc/row, ~1.8e-4 rel err,
# measured).  fp32r operands must be *produced* as fp32r (walrus
# verifier), so the attention-side tiles are declared float32r and their
# producing copy/activation instructions perform the rounding.
RD = F32R   # relaxed dtype for the attention path
# Projections in f32r too: q/k feed the gate cumsum, so this raises the
# a_k error (~sqrt(S) amplification) -- validated against the reference
# end-to-end before being kept.
PROJ_RELAXED = True
PD = F32R if PROJ_RELAXED else F32


def _build_program(with_bias: bool, reps: int = 1):
    nc = bacc.Bacc("TRN2", target_bir_lowering=False, debug=False,
                   num_devices=NCORES)

    xta_d = nc.dram_tensor("xta", [E, S], PD, kind="ExternalInput")
    wq_d = nc.dram_tensor("wq", [E + 1, NH * D], PD, kind="ExternalInput")
    wk_d = nc.dram_tensor("wk", [E + 1, NH * D], PD, kind="ExternalInput")
    wv_d = nc.dram_tensor("wv", [E + 1, NH * D], PD, kind="ExternalInput")
    wo_d = nc.dram_tensor("wo", [NH * D, E], RD, kind="ExternalInput")
    fcol_d = nc.dram_tensor("fcol", [128, 1], F32, kind="ExternalInput")
    sgns_d = nc.dram_tensor("sgns", [128, 1], F32, kind="ExternalInput")
    blka_d = nc.dram_tensor("blka", [128, 4], F32, kind="ExternalInput")
    blkb_d = nc.dram_tensor("blkb", [128, 4], F32, kind="ExternalInput")
    maskb_d = nc.dram_tensor("maskb", [128, 384], RD, kind="ExternalInput")
    out_d = nc.dram_tensor("outp", [E, S], F32, kind="ExternalOutput")

    KB = E // 128  # 8 contraction blocks of 128
    TB = S // 128  # 16 token blocks

    with tile.TileContext(nc) as tc:
        with tc.tile_pool(name="pers", bufs=1) as pers, \
             tc.tile_pool(name="const", bufs=1) as cst:
            # ---- constants ----
            fcol = cst.tile([128, 1], F32, tag="fcol", name="fcol")
            nc.sync.dma_start(fcol[:], fcol_d.ap())
            sgns = cst.tile([128, 1], F32, tag="sgns", name="sgns")
            nc.sync.dma_start(sgns[:], sgns_d.ap())
            blka = cst.tile([128, 4], F32, tag="blka", name="blka")
            nc.sync.dma_start(blka[:], blka_d.ap())
            blkb = cst.tile([128, 4], F32, tag="blkb", name="blkb")
            nc.sync.dma_start(blkb[:], blkb_d.ap())
            maskb = cst.tile([128, 384], RD, tag="maskb", name="maskb")
            nc.sync.dma_start(maskb[:], maskb_d.ap())
            ones4 = cst.tile([128, 4], F32, tag="ones4", name="ones4")
            nc.vector.memset(ones4[:], 1.0)
            ones_t = cst.tile([1, 512], F32, tag="ones_t", name="ones_t")
            nc.vector.memset(ones_t[:], 1.0)
            ones_p = cst.tile([1, 512], PD, tag="ones_p", name="ones_p")
            nc.vector.tensor_copy(ones_p[:], ones_t[:])

            for rep in range(reps):
                # ---- persistent tiles (live through attention) ----
                v_sb = []
                for tb in range(TB):
                    t = pers.tile([128, NH, D + 1], RD, tag=f"v{tb}", name=f"v{tb}")
                    v_sb.append(t)
                qR = [pers.tile([128, S], RD, tag=f"qR{p}", name=f"qR{p}")
                      for p in range(2)]
                kR = [pers.tile([128, S], RD, tag=f"kR{p}", name=f"kR{p}")
                      for p in range(2)]

                with tc.tile_pool(name=f"qkt{rep}", bufs=1) as qkt:
                    qT = [qkt.tile([128, S], PD, tag=f"qT{p}", name=f"qT{p}")
                          for p in range(2)]
                    kT = [qkt.tile([128, S], PD, tag=f"kT{p}", name=f"kT{p}")
                          for p in range(2)]
                    gates = qkt.tile([NH, S], F32, tag="gates", name="gates")
                    ak = qkt.tile([NH, S], F32, tag="ak", name="ak")

                    # ================= projections =================
                    with tc.tile_pool(name=f"xw{rep}", bufs=1) as xw:
                        xt = []
                        for kb in range(KB):
                            t = xw.tile([128, S], PD, tag=f"x{kb}", name=f"x{kb}")
                            nc.sync.dma_start(t[:], xta_d.ap()[kb * 128:(kb + 1) * 128, :])
                            xt.append(t)

                        with tc.tile_pool(name=f"wp{rep}", bufs=1) as wp, \
                             tc.tile_pool(name=f"pps{rep}", bufs=4, space="PSUM") as pps:
                            for nm, d_t, dst in (("q", wq_d, qT), ("k", wk_d, kT),
                                                 ("v", wv_d, None)):
                                wt = []
                                for kb in range(KB):
                                    t = wp.tile([128, NH * D], PD, tag=f"wt{kb}",
                                                name=f"w{nm}{kb}")
                                    nc.sync.dma_start(
                                        t[:], d_t.ap()[kb * 128:(kb + 1) * 128, :])
                                    wt.append(t)
                                wl = None
                                if with_bias:
                                    wl = wp.tile([1, NH * D], PD, tag="wl",
                                                 name=f"w{nm}L")
                                    nc.sync.dma_start(wl[:], d_t.ap()[E:E + 1, :])
                                if nm != "v":
                                    # q, k: out[feat 128, tok 512] = W^T x
                                    for p in range(2):
                                        for cx in range(4):
                                            ps = pps.tile([128, 512], F32, tag="ps",
                                                          name="ps")
                                            cs = slice(cx * 512, (cx + 1) * 512)
                                            ms = slice(p * 128, (p + 1) * 128)
                                            for kb in range(KB):
                                                nc.tensor.matmul(
                                                    ps[:], wt[kb][:, ms], xt[kb][:, cs],
                                                    start=(kb == 0),
                                                    stop=(kb == KB - 1 and not with_bias))
                                            if with_bias:
                                                nc.tensor.matmul(
                                                    ps[:], wl[0:1, ms], ones_t[0:1, :],
                                                    start=False, stop=True)
                                            nc.scalar.activation(
                                                dst[p][:, cs], ps[:],
                                                mybir.ActivationFunctionType.Copy)
                                else:
                                    # v: out[tok 128, feat 256] = x^T^T wv
                                    for tb in range(TB):
                                        ps = pps.tile([128, NH * D], F32, tag="psv",
                                                      name="psv")
                                        ts_ = slice(tb * 128, (tb + 1) * 128)
                                        for kb in range(KB):
                                            nc.tensor.matmul(
                                                ps[:], xt[kb][:, ts_],
                                                wt[kb][:],
                                                start=(kb == 0),
                                                stop=(kb == KB - 1 and not with_bias))
                                        if with_bias:
                                            nc.tensor.matmul(
                                                ps[:], ones_p[0:1, 0:128],
                                                wl[:],
                                                start=False, stop=True)
                                        nc.vector.tensor_copy(
                                            v_sb[tb][:, :, 0:D],
                                            ps[:].rearrange("p (h d) -> p h d", h=NH))
                                        nc.vector.tensor_copy(
                                            v_sb[tb][:, :, D:D + 1],
                                            ones4[:].rearrange(
                                                "p (h o) -> p h o", o=1))

                    # ============== gates + cumsum ==============
                    # gate logits for all 4 heads land on psum partitions 0-3
                    # via two accumulating matmuls with complementary
                    # block-masked ones (pair 0 -> cols 0,1; pair 1 -> 2,3).
                    with tc.tile_pool(name=f"gtmp{rep}", bufs=1) as gtmp, \
                         tc.tile_pool(name=f"gps{rep}", bufs=2, space="PSUM") as gpsp:
                        prods = []
                        for p in range(2):
                            prod = gtmp.tile([128, S], F32, tag=f"prod{p}",
                                             name=f"prod{p}")
                            nc.vector.tensor_mul(prod[:], qT[p][:], kT[p][:])
                            prods.append(prod)
                        for cx in range(4):
                            cs = slice(cx * 512, (cx + 1) * 512)
                            gps = gpsp.tile([NH, 512], F32, tag="gps", name="gps")
                            nc.tensor.matmul(
                                gps[:], blka[:],
                                prods[0][:, cs],
                                start=True, stop=False)
                            nc.tensor.matmul(
                                gps[:], blkb[:],
                                prods[1][:, cs],
                                start=False, stop=True)
                            nc.scalar.activation(
                                gates[:, cs], gps[:],
                                mybir.ActivationFunctionType.Sigmoid)
                    nc.vector.tensor_tensor_scan(
                        out=ak[:], data0=gates[:], data1=gates[:], initial=0.0,
                        op0=mybir.AluOpType.add, op1=mybir.AluOpType.bypass)

                    # ================= rope =================
                    with tc.tile_pool(name=f"rtmp{rep}", bufs=2) as rt, \
                         tc.tile_pool(name=f"trig{rep}", bufs=2) as trg:
                        for p in range(2):
                            for cx in range(4):
                                cs = slice(cx * 512, (cx + 1) * 512)
                                # a_k row -> all 128 partitions: copy the row
                                # to partition 0, then broadcast (pb only writes
                                # at partition offset 0, so one per head + the
                                # theta multiply split per 64-row half).
                                tA = rt.tile([1, 512], F32, tag="tA", name="tA")
                                nc.sync.dma_start(tA[0:1, :], ak[2 * p:2 * p + 1, cs])
                                tB = rt.tile([1, 512], F32, tag="tB", name="tB")
                                nc.sync.dma_start(tB[0:1, :], ak[2 * p + 1:2 * p + 2, cs])
                                akbA = rt.tile([128, 512], F32, tag="akbA", name="akbA")
                                nc.gpsimd.partition_broadcast(akbA[:], tA[0:1, :])
                                akbB = rt.tile([128, 512], F32, tag="akbB", name="akbB")
                                nc.gpsimd.partition_broadcast(akbB[:], tB[0:1, :])
                                # range reduction: u = a_k * f/(2pi);
                                # n = round(u) (f32->i32 copy rounds to nearest);
                                # fr = u - n in [-1/2, 1/2];
                                # sin(theta) = Sin(2pi*fr), cos via +0.25 shift.
                                u1 = rt.tile([128, 512], F32, tag="u1", name="u1")
                                nc.vector.tensor_scalar(
                                    out=u1[0:64, :], in0=akbA[0:64, :],
                                    scalar1=fcol[0:64, 0:1],
                                    scalar2=None, op0=mybir.AluOpType.mult)
                                nc.vector.tensor_scalar(
                                    out=u1[64:128, :], in0=akbB[64:128, :],
                                    scalar1=fcol[64:128, 0:1],
                                    scalar2=None, op0=mybir.AluOpType.mult)
                                u2 = rt.tile([128, 512], F32, tag="u2", name="u2")
                                nc.vector.tensor_scalar(
                                    out=u2[0:64, :], in0=akbA[0:64, :],
                                    scalar1=fcol[0:64, 0:1],
                                    scalar2=0.25, op0=mybir.AluOpType.mult,
                                    op1=mybir.AluOpType.add)
                                nc.vector.tensor_scalar(
                                    out=u2[64:128, :], in0=akbB[64:128, :],
                                    scalar1=fcol[64:128, 0:1],
                                    scalar2=0.25, op0=mybir.AluOpType.mult,
                                    op1=mybir.AluOpType.add)
                                n1 = rt.tile([128, 512], I32, tag="n1", name="n1")
                                nc.vector.tensor_copy(n1[:], u1[:])
                                n2 = rt.tile([128, 512], I32, tag="n2", name="n2")
                                nc.vector.tensor_copy(n2[:], u2[:])
                                fr1 = rt.tile([128, 512], F32, tag="fr1", name="fr1")
                                nc.vector.tensor_tensor(
                                    out=fr1[:], in0=u1[:], in1=n1[:],
                                    op=mybir.AluOpType.subtract)
                                fr2 = rt.tile([128, 512], F32, tag="fr2", name="fr2")
                                nc.vector.tensor_tensor(
                                    out=fr2[:], in0=u2[:], in1=n2[:],
                                    op=mybir.AluOpType.subtract)
                                # Ssg rows 0:32 = -sin, 32:64 = +sin (repeating)
                                Ssg = trg.tile([128, 512], F32, tag="S", name="Ssg")
                                nc.scalar.activation(
                                    Ssg[:], fr1[:], mybir.ActivationFunctionType.Sin,
                                    bias=0.0, scale=sgns[:, 0:1])
                                Cos = trg.tile([128, 512], F32, tag="C", name="Cos")
                                nc.scalar.activation(
                                    Cos[:], fr2[:], mybir.ActivationFunctionType.Sin,
                                    bias=0.0, scale=TWO_PI)
                                for src, dst in ((qT, qR), (kT, kR)):
                                    sw = rt.tile([128, 512], F32, tag="sw", name="sw")
                                    for hh in range(2):
                                        o = 64 * hh
                                        nc.gpsimd.tensor_copy(
                                            sw[o:o + 32, :],
                                            src[p][o + 32:o + 64, cs])
                                        nc.gpsimd.tensor_copy(
                                            sw[o + 32:o + 64, :],
                                            src[p][o:o + 32, cs])
                                    p1 = rt.tile([128, 512], F32, tag="p1", name="p1")
                                    nc.vector.tensor_mul(p1[:], src[p][:, cs], Cos[:])
                                    p2 = rt.tile([128, 512], F32, tag="p2", name="p2")
                                    nc.vector.tensor_mul(p2[:], sw[:], Ssg[:])
                                    nc.vector.tensor_add(dst[p][:, cs], p1[:], p2[:])

                # scheduler fence: keep every rope Sin ahead of every
                # attention Exp in the ACT stream (table-set grouping);
                # no semaphores, engines still run dataflow.
                tc.no_sync_barrier()

                # ================= attention =================
                with tc.tile_pool(name=f"late{rep}", bufs=1) as late:
                    onT = [late.tile([128, S], RD, tag=f"onT{p}", name=f"onT{p}")
                           for p in range(2)]
                    wo_sb = [late.tile([128, E], RD, tag=f"wo{p}", name=f"wo{p}")
                             for p in range(2)]
                    for p in range(2):
                        nc.sync.dma_start(
                            wo_sb[p][:], wo_d.ap()[p * 128:(p + 1) * 128, :])

                    with tc.tile_pool(name=f"qkps{rep}", bufs=3, space="PSUM") as qkps, \
                         tc.tile_pool(name=f"pvps{rep}", bufs=2, space="PSUM") as pvps, \
                         tc.tile_pool(name=f"ptp{rep}", bufs=4) as ptp, \
                         tc.tile_pool(name=f"nrm{rep}", bufs=4) as nrm:
                        for h in range(NH):
                            p, half = h // 2, h % 2
                            rows = slice(64 * half, 64 * half + 64)
                            qh = qR[p][rows, :]
                            kh = kR[p][rows, :]
                            for sb in range(NSLAB):
                                i0 = sb * SLAB
                                isl = slice(i0, i0 + SLAB)
                                njb = 2 * sb + 2
                                ov = pvps.tile([D + 1, SLAB], F32, tag="ov", name="ov")
                                jb0 = 0
                                while jb0 < njb:
                                    g = min(GROUP, njb - jb0)
                                    ps = qkps.tile([128, GROUP * SLAB], F32,
                                                   tag="qk", name="qk")
                                    for i in range(g):
                                        jb = jb0 + i
                                        nc.tensor.matmul(
                                            ps[:, i * SLAB:(i + 1) * SLAB],
                                            kh[:, jb * 128:(jb + 1) * 128],
                                            qh[:, isl],
                                            start=True, stop=True)
                                    pt = ptp.tile([128, GROUP * SLAB], RD,
                                                  tag="pt", name="pt")
                                    nc.scalar.activation(
                                        pt[:, 0:g * SLAB], ps[:, 0:g * SLAB],
                                        mybir.ActivationFunctionType.Exp)
                                    for i in range(g):
                                        jb = jb0 + i
                                        r = jb - 2 * sb
                                        if r >= 0:  # diagonal: causal 0/1 mask
                                            msl = maskb[:, 128 - 128 * r:384 - 128 * r]
                                            nc.vector.tensor_mul(
                                                pt[:, i * SLAB:(i + 1) * SLAB],
                                                pt[:, i * SLAB:(i + 1) * SLAB], msl)
                                    for i in range(g):
                                        jb = jb0 + i
                                        nc.tensor.matmul(
                                            ov[:], v_sb[jb][:, h, :],
                                            pt[:, i * SLAB:(i + 1) * SLAB],
                                            start=(jb == 0), stop=(jb == njb - 1))
                                    jb0 += g
                                rc = nrm.tile([1, SLAB], F32, tag="rc", name="rc")
                                nc.vector.reciprocal(rc[:], ov[D:D + 1, :])
                                rcb = nrm.tile([64, SLAB], F32, tag="rcb", name="rcb")
                                nc.gpsimd.partition_broadcast(rcb[:], rc[:])
                                nc.vector.tensor_mul(
                                    onT[p][rows, isl], ov[0:D, :], rcb[:])

                    # ================= output projection =================
                    with tc.tile_pool(name=f"ops{rep}", bufs=4, space="PSUM") as opsp, \
                         tc.tile_pool(name=f"osb{rep}", bufs=4) as osbp:
                        for mb in range(E // 128):
                            for cx in range(4):
                                cs = slice(cx * 512, (cx + 1) * 512)
                                ms = slice(mb * 128, (mb + 1) * 128)
                                ps = opsp.tile([128, 512], F32, tag="ops", name="ops")
                                for p in range(2):
                                    nc.tensor.matmul(
                                        ps[:], wo_sb[p][:, ms],
                                        onT[p][:, cs],
                                        start=(p == 0), stop=(p == 1))
                                ob = osbp.tile([128, 512], F32, tag="ob", name="ob")
                                nc.scalar.activation(
                                    ob[:], ps[:],
                                    mybir.ActivationFunctionType.Copy)
                                nc.sync.dma_start(out_d.ap()[ms, cs], ob[:])

    nc.compile()
    return nc


_PROGRAMS: dict = {}


def _get_program(with_bias: bool, reps: int = 1):
    key = (with_bias, reps)
    if key not in _PROGRAMS:
        _PROGRAMS[key] = _build_program(with_bias, reps)
    return _PROGRAMS[key]


def _host_consts():
    freqs = THETA ** (-np.arange(32, dtype=np.float64) / 32.0)
    fcol = np.tile(freqs / TWO_PI, 4).astype(np.float32).reshape(128, 1)
    sgns = np.tile(np.repeat(np.float32([-TWO_PI, TWO_PI]), 32), 2).reshape(128, 1)
    blka = np.zeros((128, 4), np.float32)
    blka[0:64, 0] = 1.0
    blka[64:128, 1] = 1.0
    blkb = np.zeros((128, 4), np.float32)
    blkb[0:64, 2] = 1.0
    blkb[64:128, 3] = 1.0
    jp = np.arange(128)[:, None]
    u = np.arange(384)[None, :]
    maskb = (jp <= u - 128).astype(np.float32)
    return fcol, sgns, blka, blkb, maskb


def _prepare_in_maps(inputs):
    x = np.ascontiguousarray(np.asarray(inputs["x"], np.float32))
    Wq, bq = np.asarray(inputs["Wq"], np.float32), np.asarray(inputs["bq"], np.float32)
    Wk, bk = np.asarray(inputs["Wk"], np.float32), np.asarray(inputs["bk"], np.float32)
    Wv, bv = np.asarray(inputs["Wv"], np.float32), np.asarray(inputs["bv"], np.float32)
    Wo = np.asarray(inputs["Wo"], np.float32)

    fcol, sgns, blka, blkb, maskb = _host_consts()

    in_maps = []
    for c in range(NCORES):
        b, hg = c // 4, c % 4
        heads = [4 * hg + j for j in range(4)]
        permQ = []
        for p in range(2):
            for hh in (heads[2 * p], heads[2 * p + 1]):
                permQ.extend(hh * D + 2 * np.arange(32))
                permQ.extend(hh * D + 2 * np.arange(32) + 1)
        permQ = np.asarray(permQ)
        permV = np.concatenate([hh * D + np.arange(D) for hh in heads])

        xta = np.ascontiguousarray(x[b].T)
        wq = np.empty((E + 1, NH * D), np.float32)
        wq[0:E] = (Wq[permQ] * SM).T
        wq[E] = bq[permQ] * SM
        wk = np.empty((E + 1, NH * D), np.float32)
        wk[0:E] = Wk[permQ].T
        wk[E] = bk[permQ]
        wv = np.empty((E + 1, NH * D), np.float32)
        wv[0:E] = Wv[permV].T
        wv[E] = bv[permV]
        wo = np.ascontiguousarray(Wo[:, permV].T)
        in_maps.append({
            "xta": xta, "wq": wq, "wk": wk, "wv": wv, "wo": wo,
            "fcol": fcol, "sgns": sgns, "blka": blka,
            "blkb": blkb, "maskb": maskb,
        })
    return in_maps


def kernel(x, Wq, bq, Wk, bk, Wv, bv, Wo, bo):
    inputs = {"x": x, "Wq": Wq, "bq": bq, "Wk": Wk, "bk": bk,
              "Wv": Wv, "bv": bv, "Wo": Wo, "bo": bo}
    bq_, bk_, bv_ = (np.asarray(b, np.float32) for b in (bq, bk, bv))
    bo_ = np.asarray(bo, np.float32)
    with_bias = bool(np.any(bq_) or np.any(bk_) or np.any(bv_))
    nc = _get_program(with_bias)
    in_maps = _prepare_in_maps(inputs)

    res = run_bass_kernel_spmd(nc, in_maps, core_ids=list(range(NCORES)))

    out = np.empty((B, S, E), np.float32)
    for b in range(B):
        acc = res.results[4 * b]["outp"].astype(np.float64)
        for c in range(4 * b + 1, 4 * b + 4):
            acc = acc + res.results[c]["outp"]
        out[b] = acc.T + bo_
    return out


if __name__ == "__main__":
    rng = np.random.default_rng(0)
    ins = {
        "x": rng.standard_normal((B, S, E)).astype(np.float32),
        "Wq": (rng.standard_normal((E, E)) * 0.02).astype(np.float32),
        "bq": np.zeros(E, np.float32),
        "Wk": (rng.standard_normal((E, E)) * 0.02).astype(np.float32),
        "bk": np.zeros(E, np.float32),
        "Wv": (rng.standard_normal((E, E)) * 0.02).astype(np.float32),
        "bv": np.zeros(E, np.float32),
        "Wo": (rng.standard_normal((E, E)) * 0.02).astype(np.float32),
        "bo": np.zeros(E, np.float32),
    }
    o = kernel(**ins)
    print("kernel ran, out", o.shape, o.dtype, float(np.abs(o).max()))

